# Initial kernel scaffold
#
"""Point-transformer block kernel for TRN2 (8-core data-parallel).

Core i handles serialized patches 2i,2i+1: rows = order[2048i:2048(i+1)].
CPE neighbor gather: dma_gather(transpose=True) from replicated bf16 feat
-> feature-major gathered tiles. Activations feature-major (FM):
X^T [128 (c%128), CC (c//128), rows]; matmuls lhsT=W^T-arranged weights.
V row-major via activation-stationary matmuls. bf16 matmuls, f32 residual.
"""
from contextlib import ExitStack

import numpy as np
import ml_dtypes

import concourse.bacc as bacc
import concourse.bass as bass
import concourse.mybir as mybir
import concourse.tile as tile

P = 128
C = 512
CC = C // P
NH = 8
HD = 64
KP = 1024
R = 2048
NPATCH = R // KP
NKK = 27
NFULL = 16384
EPS = 1e-5
SCALE = (C // NH) ** -0.5
F32 = mybir.dt.float32
BF16 = mybir.dt.bfloat16
I16 = mybir.dt.int16
AF = mybir.ActivationFunctionType
OP = mybir.AluOpType

HALF = 1024
NHALF = R // HALF
N512 = HALF // 512


def build_program(gelu_exact=True):
    nc = bacc.Bacc("TRN2", target_bir_lowering=False, debug=False)

    feat_bf = nc.dram_tensor("feat_bf", [NFULL + 1, C], BF16, kind="ExternalInput")
    featT_own = nc.dram_tensor("featT_own", [P, CC, R], F32, kind="ExternalInput")
    gidx = nc.dram_tensor("gidx", [NHALF * NKK, P, HALF // 16], I16,
                          kind="ExternalInput")

    wcat = nc.dram_tensor("wcat", [NKK, C, C], BF16, kind="ExternalInput")
    lin_wT = nc.dram_tensor("lin_wT", [C, C], BF16, kind="ExternalInput")
    qkv_wT = nc.dram_tensor("qkv_wT", [C, 3 * C], BF16, kind="ExternalInput")
    proj_wT = nc.dram_tensor("proj_wT", [C, C], BF16, kind="ExternalInput")
    fc1_wT = nc.dram_tensor("fc1_wT", [C, 4 * C], BF16, kind="ExternalInput")
    fc2_wT = nc.dram_tensor("fc2_wT", [4 * C, C], BF16, kind="ExternalInput")

    pvecs = {}
    for nm, n in [("cpe_b", CC), ("lin_b", CC), ("cpe_ln_g", CC), ("cpe_ln_b", CC),
                  ("ln1_g", CC), ("ln1_b", CC), ("ln2_g", CC), ("ln2_b", CC),
                  ("q_b", CC), ("k_b", CC), ("proj_b", CC),
                  ("fc1_b", 4 * CC), ("fc2_b", CC)]:
        pvecs[nm] = nc.dram_tensor(nm, [P, n], F32, kind="ExternalInput")
    v_b_rep = nc.dram_tensor("v_b_rep", [P, C], F32, kind="ExternalInput")

    outT = nc.dram_tensor("outT", [P, CC, R], F32, kind="ExternalOutput")

    with tile.TileContext(nc) as tc, ExitStack() as ctx:
        pers = ctx.enter_context(tc.tile_pool(name="pers", bufs=1))
        resid = ctx.enter_context(tc.tile_pool(name="resid", bufs=2))

        pv = {}
        for nm, dram in pvecs.items():
            t = pers.tile(list(dram.shape), F32, tag=f"pv_{nm}")
            nc.sync.dma_start(t[:], dram[:])
            pv[nm] = t
        v_b_t = pers.tile([P, C], F32, tag="v_b")
        nc.sync.dma_start(v_b_t[:], v_b_rep[:])
        ones_bf = pers.tile([P, P], BF16, tag="ones_bf")
        nc.vector.memset(ones_bf[:], 1.0)
        ones_f = pers.tile([P, P], F32, tag="ones_f")
        nc.vector.memset(ones_f[:], 1.0)
        ones1_f = pers.tile([1, HD], F32, tag="ones1_f")
        nc.vector.memset(ones1_f[:], 1.0)
        eps_t = pers.tile([P, 1], F32, tag="eps_t")
        nc.vector.memset(eps_t[:], EPS)

        def wload(pool, dram_ap, kdim, ndim, tag):
            t = pool.tile([P, kdim // P, ndim], BF16, tag=tag)
            nc.sync.dma_start(t[:], dram_ap.rearrange("(ko ki) n -> ki ko n", ki=P))
            return t

        def fm_ln_stats(lnp, x, x_is_f32):
            with tc.tile_pool(name="ln_ps", bufs=1, space="PSUM") as lps:
                sums_ps = lps.tile([P, R], F32, tag="ln_sums")
                sqs_ps = lps.tile([P, R], F32, tag="ln_sqs")
                for half in range(NHALF):
                    o = half * HALF
                    sq = lnp.tile([P, CC, HALF], BF16, tag="ln_sq")
                    nc.scalar.activation(sq[:], x[:, :, o:o + HALF], AF.Square)
                    if x_is_f32:
                        # bf16 shadow so the sums matmul runs at 1 cyc/row
                        xb = lnp.tile([P, CC, HALF], BF16, tag="ln_xb")
                        nc.vector.tensor_copy(xb[:], x[:, :, o:o + HALF])
                    for kc in range(CC):
                        for nn in range(N512):
                            sl = slice(o + nn * 512, o + (nn + 1) * 512)
                            sli = slice(nn * 512, (nn + 1) * 512)
                            xsrc = xb[:, kc, sli] if x_is_f32 else x[:, kc, sl]
                            nc.tensor.matmul(sums_ps[:, sl], ones_bf[:], xsrc,
                                             start=(kc == 0), stop=(kc == CC - 1))
                            nc.tensor.matmul(sqs_ps[:, sl], ones_bf[:],
                                             sq[:, kc, sli],
                                             start=(kc == 0), stop=(kc == CC - 1))
                neg_m = lnp.tile([P, R], F32, tag="ln_negm")
                nc.vector.tensor_scalar(neg_m[:], sums_ps[:], -1.0 / C, None,
                                        op0=OP.mult)
                msq = lnp.tile([P, R], F32, tag="ln_tmp")
                nc.scalar.activation(msq[:], neg_m[:], AF.Square)
                var = lnp.tile([P, R], F32, tag="ln_tmp2")
                nc.vector.scalar_tensor_tensor(var[:], sqs_ps[:], 1.0 / C, msq[:],
                                               op0=OP.mult, op1=OP.subtract)
            std = lnp.tile([P, R], F32, tag="ln_tmp")
            nc.scalar.activation(std[:], var[:], AF.Sqrt, bias=eps_t[:])
            inv_std = lnp.tile([P, R], F32, tag="ln_istd")
            nc.vector.reciprocal(inv_std[:], std[:])
            return neg_m, inv_std

        def fm_ln_apply(lnp, x, neg_m, inv_std, g, b, out, m, res=None):
            t = lnp.tile([P, R], F32, tag="ln_t")
            nc.vector.tensor_tensor(t[:], x[:, m, :], neg_m[:], op=OP.add)
            nc.vector.tensor_tensor(t[:], t[:], inv_std[:], op=OP.mult)
            if res is None:
                nc.vector.tensor_scalar(out[:, m, :], t[:], g[:, m:m + 1],
                                        b[:, m:m + 1], op0=OP.mult, op1=OP.add)
            else:
                nc.vector.tensor_scalar(t[:], t[:], g[:, m:m + 1], b[:, m:m + 1],
                                        op0=OP.mult, op1=OP.add)
                nc.vector.tensor_tensor(out[:, m, :], t[:], res[:, m, :], op=OP.add)

        feat1 = None

        # =========================== CPE =================================
        with tc.tile_pool(name="hpool", bufs=1) as hpool:
            h1_sb = hpool.tile([P, CC, R], BF16, tag="h1")
            with (
                tc.tile_pool(name="cpe_in", bufs=3) as cin,
                tc.tile_pool(name="cpe_ps", bufs=1, space="PSUM") as cps,
            ):
                for half in range(NHALF):
                    hp = [cps.tile([P, HALF], F32, tag=f"mm_ps{m}", name=f"cpe_ps{m}") for m in range(CC)]
                    for kk in range(NKK):
                        w_t = wload(cin, wcat[kk], C, C, "wkk")
                        gi = cin.tile([P, HALF // 16], I16, tag="gi")
                        nc.sync.dma_start(gi[:], gidx[half * NKK + kk])
                        g_t = cin.tile([P, CC, HALF], BF16, tag="gt")
                        nc.gpsimd.dma_gather(g_t[:], feat_bf[:], gi[:], HALF, HALF,
                                             C, transpose=True, single_packet=False)
                        for kc in range(CC):
                            for m in range(CC):
                                for nn in range(N512):
                                    sl = slice(nn * 512, (nn + 1) * 512)
                                    nc.tensor.matmul(
                                        hp[m][:, sl],
                                        w_t[:, kc, m * P:(m + 1) * P],
                                        g_t[:, kc, sl],
                                        start=(kk == 0 and kc == 0),
                                        stop=(kk == NKK - 1 and kc == CC - 1))
                    for m in range(CC):
                        nc.vector.tensor_scalar(
                            h1_sb[:, m, half * HALF:(half + 1) * HALF], hp[m][:],
                            pv["cpe_b"][:, m:m + 1], None, op0=OP.add)

            h2_sb = hpool.tile([P, CC, R], BF16, tag="h2")
            with (
                tc.tile_pool(name="linw", bufs=1) as lwp,
                tc.tile_pool(name="lin_ps", bufs=1, space="PSUM") as lps,
            ):
                lin_w_t = wload(lwp, lin_wT[:], C, C, "lin_w")
                for half in range(NHALF):
                    hp = [lps.tile([P, HALF], F32, tag=f"mm_ps{m}", name=f"lin_ps{m}") for m in range(CC)]
                    for kc in range(CC):
                        for m in range(CC):
                            for nn in range(N512):
                                sl = slice(nn * 512, (nn + 1) * 512)
                                hsl = slice(half * HALF + nn * 512,
                                            half * HALF + (nn + 1) * 512)
                                nc.tensor.matmul(
                                    hp[m][:, sl],
                                    lin_w_t[:, kc, m * P:(m + 1) * P],
                                    h1_sb[:, kc, hsl],
                                    start=(kc == 0), stop=(kc == CC - 1))
                    for m in range(CC):
                        nc.vector.tensor_scalar(
                            h2_sb[:, m, half * HALF:(half + 1) * HALF], hp[m][:],
                            pv["lin_b"][:, m:m + 1], None, op0=OP.add)

            feat1 = resid.tile([P, CC, R], F32, tag="resid")
            with tc.tile_pool(name="fown_p", bufs=1) as fp_, \
                 tc.tile_pool(name="lnp1", bufs=1) as lnp:
                neg_m, inv_std = fm_ln_stats(lnp, h2_sb, x_is_f32=False)
                fown = fp_.tile([P, CC, R], F32, tag="fown")
                nc.sync.dma_start(fown[:], featT_own[:])
                for m in range(CC):
                    fm_ln_apply(lnp, h2_sb, neg_m, inv_std, pv["cpe_ln_g"],
                                pv["cpe_ln_b"], feat1, m, res=fown)

        # ===================== ln1 + qkv + attn + proj ===================
        with tc.tile_pool(name="attn", bufs=1) as ap_:
            q_sb = ap_.tile([P, CC, R], BF16, tag="q_sb")
            k_sb = ap_.tile([P, CC, R], BF16, tag="k_sb")
            v_sb = ap_.tile([P, R // P, NH, HD + 1], BF16, tag="v_sb")
            o_sb = ap_.tile([P, CC, R], BF16, tag="o_sb")
            nc.vector.memset(v_sb[:], 1.0)

            with tc.tile_pool(name="x1p", bufs=1) as x1p:
                with tc.tile_pool(name="lnp2", bufs=1) as lnp:
                    neg_m, inv_std = fm_ln_stats(lnp, feat1, x_is_f32=True)
                    x1 = x1p.tile([P, CC, R], BF16, tag="x1")
                    for m in range(CC):
                        fm_ln_apply(lnp, feat1, neg_m, inv_std, pv["ln1_g"],
                                    pv["ln1_b"], x1, m)

                with tc.tile_pool(name="qkv_ps", bufs=1, space="PSUM") as qps, \
                     tc.tile_pool(name="qkvw", bufs=2) as qwp:
                    for half in range(NHALF):
                        o = half * HALF
                        for part, dst, bias, scl in [(0, q_sb, pv["q_b"], SCALE),
                                                     (1, k_sb, pv["k_b"], None)]:
                            qkv_w_t = wload(qwp, qkv_wT[:, part * C:(part + 1) * C],
                                            C, C, "qkv_w_part")
                            pp_ = [qps.tile([P, HALF], F32, tag=f"mm_ps{m}", name=f"qkv_ps{m}")
                                   for m in range(CC)]
                            for kc in range(CC):
                                for m in range(CC):
                                    for nn in range(N512):
                                        sl = slice(nn * 512, (nn + 1) * 512)
                                        nc.tensor.matmul(
                                            pp_[m][:, sl],
                                            qkv_w_t[:, kc, m * P:(m + 1) * P],
                                            x1[:, kc, o + nn * 512:
                                               o + (nn + 1) * 512],
                                            start=(kc == 0), stop=(kc == CC - 1))
                            for m in range(CC):
                                if scl is None:
                                    nc.vector.tensor_scalar(
                                        dst[:, m, o:o + HALF], pp_[m][:],
                                        bias[:, m:m + 1], None, op0=OP.add)
                                else:
                                    nc.vector.tensor_scalar(
                                        dst[:, m, o:o + HALF], pp_[m][:],
                                        bias[:, m:m + 1], scl,
                                        op0=OP.add, op1=OP.mult)
                        v_w_t = wload(qwp, qkv_wT[:, 2 * C:3 * C], C, C,
                                      "qkv_w_part")
                        for rc in range(HALF // P):
                            row0 = o + rc * P
                            vp = qps.tile([P, C], F32, tag=f"mm_ps{rc % 2}",
                                          name="vp")
                            for kc in range(CC):
                                nc.tensor.matmul(
                                    vp[:], x1[:, kc, row0:row0 + P],
                                    v_w_t[:, kc, :],
                                    start=(kc == 0), stop=(kc == CC - 1))
                            nc.vector.tensor_tensor(
                                v_sb[:, row0 // P, :, :HD],
                                vp[:].rearrange("p (h d) -> p h d", d=HD),
                                v_b_t[:].rearrange("p (h d) -> p h d", d=HD),
                                op=OP.add)

            with (
                tc.tile_pool(name="pT_pool", bufs=2) as ptp,
                tc.tile_pool(name="at_ps", bufs=1, space="PSUM") as aps,
                tc.tile_pool(name="at_ps2", bufs=2, space="PSUM") as aps2,
            ):
                for pt in range(NPATCH):
                    po = pt * KP
                    for h in range(NH):
                        hc, hpo = divmod(h * HD, P)
                        pT = ptp.tile([P, KP // P, KP], BF16, tag="pT")
                        for jc in range(KP // P):
                            sps = aps2.tile([P, KP], F32, tag="s_ps")
                            for nn in range(KP // 512):
                                nc.tensor.matmul(
                                    sps[:, nn * 512:(nn + 1) * 512],
                                    k_sb[hpo:hpo + HD, hc,
                                         po + jc * P:po + (jc + 1) * P],
                                    q_sb[hpo:hpo + HD, hc,
                                         po + nn * 512:po + (nn + 1) * 512],
                                    start=True, stop=True)
                            nc.scalar.activation(pT[:, jc, :], sps[:], AF.Exp)
                        ops_ = aps.tile([P, KP], F32, tag="o_ps")
                        for jc in range(KP // P):
                            for nn in range(KP // 512):
                                sl = slice(nn * 512, (nn + 1) * 512)
                                nc.tensor.matmul(
                                    ops_[:HD + 1, sl],
                                    v_sb[:, (po + jc * P) // P, h, :],
                                    pT[:, jc, sl],
                                    start=(jc == 0), stop=(jc == KP // P - 1))
                        # free the O psum bank early: copy to SBUF, then
                        # run the normalize tail off the PE critical path
                        ocp = ptp.tile([HD + 1, KP], F32, tag="ocp")
                        nc.scalar.copy(ocp[:], ops_[:HD + 1, :])
                        rec = ptp.tile([1, KP], F32, tag="rec")
                        nc.vector.reciprocal(rec[:], ocp[HD:HD + 1, :])
                        rps = aps.tile([HD, KP], F32, tag="rec_ps")
                        for nn in range(KP // 512):
                            sl = slice(nn * 512, (nn + 1) * 512)
                            nc.tensor.matmul(rps[:, sl], ones1_f[:], rec[:, sl],
                                             start=True, stop=True)
                        nc.vector.tensor_tensor(
                            o_sb[hpo:hpo + HD, hc, po:po + KP],
                            ocp[:HD, :], rps[:], op=OP.mult)

            feat2 = resid.tile([P, CC, R], F32, tag="resid")
            with (
                tc.tile_pool(name="projw", bufs=1) as pwp,
                tc.tile_pool(name="proj_ps", bufs=1, space="PSUM") as pps,
            ):
                proj_w_t = wload(pwp, proj_wT[:], C, C, "proj_w")
                for half in range(NHALF):
                    o = half * HALF
                    pp_ = [pps.tile([P, HALF], F32, tag=f"mm_ps{m}", name=f"proj_ps{m}")
                           for m in range(CC)]
                    for kc in range(CC):
                        for m in range(CC):
                            for nn in range(N512):
                                sl = slice(nn * 512, (nn + 1) * 512)
                                nc.tensor.matmul(
                                    pp_[m][:, sl],
                                    proj_w_t[:, kc, m * P:(m + 1) * P],
                                    o_sb[:, kc, o + nn * 512:o + (nn + 1) * 512],
                                    start=(kc == 0), stop=(kc == CC - 1))
                    for m in range(CC):
                        nc.vector.scalar_tensor_tensor(
                            feat2[:, m, o:o + HALF], pp_[m][:],
                            pv["proj_b"][:, m:m + 1], feat1[:, m, o:o + HALF],
                            op0=OP.add, op1=OP.add)

        # =============================== MLP =============================
        QH = 512  # row quarter
        with tc.tile_pool(name="mlp", bufs=1) as mp_:
            x2 = mp_.tile([P, CC, R], BF16, tag="x2")
            with tc.tile_pool(name="lnp3", bufs=1) as lnp:
                neg_m, inv_std = fm_ln_stats(lnp, feat2, x_is_f32=True)
                for m in range(CC):
                    fm_ln_apply(lnp, feat2, neg_m, inv_std, pv["ln2_g"],
                                pv["ln2_b"], x2, m)

            gelu_f = AF.Gelu if gelu_exact else AF.Tanh
            fc1_w_t = wload(mp_, fc1_wT[:], C, 4 * C, "fc1_w")
            fc2_w_t = wload(mp_, fc2_wT[:], 4 * C, C, "fc2_w")
            with (
                tc.tile_pool(name="g_pool", bufs=2) as gp_,
                tc.tile_pool(name="out_pool", bufs=2) as op_,
                tc.tile_pool(name="mlp_ps", bufs=2, space="PSUM") as mps,
            ):
                for quarter in range(R // QH):
                    o = quarter * QH
                    g_sb = gp_.tile([P, 4 * CC, QH], BF16, tag="g_sb")
                    for mg in range(4):
                        fp = [mps.tile([P, QH], F32, tag=f"mm_ps{m}", name=f"mlp_ps{m}")
                              for m in range(CC)]
                        for kc in range(CC):
                            for m in range(CC):
                                mm = mg * CC + m
                                nc.tensor.matmul(
                                    fp[m][:],
                                    fc1_w_t[:, kc, mm * P:(mm + 1) * P],
                                    x2[:, kc, o:o + QH],
                                    start=(kc == 0), stop=(kc == CC - 1))
                        for m in range(CC):
                            mm = mg * CC + m
                            nc.scalar.activation(g_sb[:, mm, :], fp[m][:], gelu_f,
                                                 bias=pv["fc1_b"][:, mm:mm + 1])
                    f2 = [mps.tile([P, QH], F32, tag=f"mm_ps{m}", name=f"mlp_ps{m}") for m in range(CC)]
                    for kc in range(4 * CC):
                        for m in range(CC):
                            nc.tensor.matmul(
                                f2[m][:],
                                fc2_w_t[:, kc, m * P:(m + 1) * P],
                                g_sb[:, kc, :],
                                start=(kc == 0), stop=(kc == 4 * CC - 1))
                    out_q = op_.tile([P, CC, QH], F32, tag="out_q")
                    for m in range(CC):
                        nc.vector.scalar_tensor_tensor(
                            out_q[:, m, :], f2[m][:], pv["fc2_b"][:, m:m + 1],
                            feat2[:, m, o:o + QH], op0=OP.add, op1=OP.add)
                    nc.sync.dma_start(outT[:, :, o:o + QH], out_q[:])

    nc.compile()
    return nc


# ====================== host-side preparation ======================

def prep_shared(inputs):
    f32 = np.float32
    bf = ml_dtypes.bfloat16

    def pp(v):
        return np.ascontiguousarray(np.asarray(v, f32).reshape(-1, P).T)

    feat = np.asarray(inputs["feat"], f32)
    feat_bf = np.zeros((NFULL + 1, C), bf)
    feat_bf[:NFULL] = feat.astype(bf)

    qkv_b = np.asarray(inputs["qkv_b"], f32)
    return dict(
        feat_bf=feat_bf,
        wcat=np.ascontiguousarray(
            np.transpose(np.asarray(inputs["cpe_w"], f32), (0, 2, 1))).astype(bf),
        lin_wT=np.ascontiguousarray(np.asarray(inputs["cpe_lin_w"], f32).T).astype(bf),
        qkv_wT=np.ascontiguousarray(np.asarray(inputs["qkv_w"], f32).T).astype(bf),
        proj_wT=np.ascontiguousarray(np.asarray(inputs["proj_w"], f32).T).astype(bf),
        fc1_wT=np.ascontiguousarray(np.asarray(inputs["fc1_w"], f32).T).astype(bf),
        fc2_wT=np.ascontiguousarray(np.asarray(inputs["fc2_w"], f32).T).astype(bf),
        cpe_b=pp(inputs["cpe_b"]), lin_b=pp(inputs["cpe_lin_b"]),
        cpe_ln_g=pp(inputs["cpe_ln_g"]), cpe_ln_b=pp(inputs["cpe_ln_b"]),
        ln1_g=pp(inputs["ln1_g"]), ln1_b=pp(inputs["ln1_b"]),
        ln2_g=pp(inputs["ln2_g"]), ln2_b=pp(inputs["ln2_b"]),
        q_b=pp(qkv_b[:C]), k_b=pp(qkv_b[C:2 * C]),
        v_b_rep=np.ascontiguousarray(np.broadcast_to(qkv_b[2 * C:], (P, C))),
        proj_b=pp(inputs["proj_b"]),
        fc1_b=pp(inputs["fc1_b"]), fc2_b=pp(inputs["fc2_b"]),
    )


def prep_core(inputs, core):
    f32 = np.float32
    order = np.asarray(inputs["order"])
    feat = np.asarray(inputs["feat"], f32)
    nbr = np.asarray(inputs["neighbor_idx"])
    rows = order[core * R:(core + 1) * R]

    featT_own = np.ascontiguousarray(
        feat[rows].T.reshape(CC, P, R).transpose(1, 0, 2))

    nb = nbr[rows].astype(np.int64)
    nb[nb < 0] = NFULL
    gidx = np.zeros((NHALF * NKK, P, HALF // 16), np.int16)
    for half in range(NHALF):
        blk = nb[half * HALF:(half + 1) * HALF]
        for kk in range(NKK):
            w = blk[:, kk].astype(np.int16).reshape(-1, 16).T
            g = gidx[half * NKK + kk]
            for rep in range(P // 16):
                g[rep * 16:(rep + 1) * 16] = w
    return dict(featT_own=featT_own, gidx=gidx), rows


def unshard_out(res_outT):
    return np.ascontiguousarray(
        np.asarray(res_outT).transpose(1, 0, 2).reshape(C, R).T)


# ======================= public entry point =======================

_CACHED_NC = None


def _get_nc():
    global _CACHED_NC
    if _CACHED_NC is None:
        _CACHED_NC = build_program(gelu_exact=True)
    return _CACHED_NC


def kernel(**inputs) -> np.ndarray:
    """Full-input, full-output entry. Shards across 8 NeuronCores by
    serialized patches (2 per core), runs the Bass kernel, scatters the
    per-core outputs back to original point order."""
    from concourse.bass_utils import run_bass_kernel_spmd

    nc = _get_nc()
    inputs = {k: np.asarray(v) for k, v in inputs.items()}
    sh = prep_shared(inputs)
    in_maps, rows_l = [], []
    for c in range(8):
        ci, rows = prep_core(inputs, c)
        in_maps.append({**sh, **ci})
        rows_l.append(rows)

    res = None
    last_err = None
    for attempt in range(3):
        try:
            res = run_bass_kernel_spmd(nc, in_maps, core_ids=list(range(8))).results
            break
        except Exception as e:   # transient NRT/axon hiccups: retry
            last_err = e
            import time as _t
            _t.sleep(2.0)
    if res is None:
        raise last_err

    out = np.zeros((NFULL, C), np.float32)
    for c in range(8):
        out[rows_l[c]] = unshard_out(res[c]["outT"])
    return out



# revision 1
# speedup vs baseline: 17.5114x; 17.5114x over previous
"""Point-transformer block kernel for TRN2 (8-core data-parallel).

Core i handles serialized patches 2i,2i+1: rows = order[2048i:2048(i+1)].
CPE neighbor gather: dma_gather(transpose=True) from replicated bf16 feat
-> feature-major gathered tiles. Activations feature-major (FM):
X^T [128 (c%128), CC (c//128), rows]; matmuls lhsT=W^T-arranged weights.
V row-major via activation-stationary matmuls. bf16 matmuls, f32 residual.
"""
from contextlib import ExitStack

import numpy as np
import ml_dtypes

import concourse.bacc as bacc
import concourse.bass as bass
import concourse.mybir as mybir
import concourse.tile as tile

P = 128
C = 512
CC = C // P
NH = 8
HD = 64
KP = 1024
R = 2048
NPATCH = R // KP
NKK = 27
NFULL = 16384
EPS = 1e-5
SCALE = (C // NH) ** -0.5
F32 = mybir.dt.float32
BF16 = mybir.dt.bfloat16
I16 = mybir.dt.int16
AF = mybir.ActivationFunctionType
OP = mybir.AluOpType

HALF = 1024
NHALF = R // HALF
N512 = HALF // 512


def build_program(gelu_exact=True):
    nc = bacc.Bacc("TRN2", target_bir_lowering=False, debug=False)

    feat_bf = nc.dram_tensor("feat_bf", [NFULL + 1, C], BF16, kind="ExternalInput")
    featT_own = nc.dram_tensor("featT_own", [P, CC, R], F32, kind="ExternalInput")
    gidx = nc.dram_tensor("gidx", [NHALF * NKK, P, HALF // 16], I16,
                          kind="ExternalInput")

    wcat = nc.dram_tensor("wcat", [NKK, C, C], BF16, kind="ExternalInput")
    lin_wT = nc.dram_tensor("lin_wT", [C, C], BF16, kind="ExternalInput")
    qkv_wT = nc.dram_tensor("qkv_wT", [C, 3 * C], BF16, kind="ExternalInput")
    proj_wT = nc.dram_tensor("proj_wT", [C, C], BF16, kind="ExternalInput")
    fc1_wT = nc.dram_tensor("fc1_wT", [C, 4 * C], BF16, kind="ExternalInput")
    fc2_wT = nc.dram_tensor("fc2_wT", [4 * C, C], BF16, kind="ExternalInput")

    pvecs = {}
    for nm, n in [("cpe_b", CC), ("lin_b", CC), ("cpe_ln_g", CC), ("cpe_ln_b", CC),
                  ("ln1_g", CC), ("ln1_b", CC), ("ln2_g", CC), ("ln2_b", CC),
                  ("q_b", CC), ("k_b", CC), ("proj_b", CC),
                  ("fc1_b", 4 * CC), ("fc2_b", CC)]:
        pvecs[nm] = nc.dram_tensor(nm, [P, n], F32, kind="ExternalInput")
    v_b_rep = nc.dram_tensor("v_b_rep", [P, C], F32, kind="ExternalInput")

    outT = nc.dram_tensor("outT", [P, CC, R], F32, kind="ExternalOutput")

    with tile.TileContext(nc) as tc, ExitStack() as ctx:
        pers = ctx.enter_context(tc.tile_pool(name="pers", bufs=1))
        resid = ctx.enter_context(tc.tile_pool(name="resid", bufs=2))

        pv = {}
        for nm, dram in pvecs.items():
            t = pers.tile(list(dram.shape), F32, tag=f"pv_{nm}")
            nc.sync.dma_start(t[:], dram[:])
            pv[nm] = t
        v_b_t = pers.tile([P, C], F32, tag="v_b")
        nc.sync.dma_start(v_b_t[:], v_b_rep[:])
        ones_bf = pers.tile([P, P], BF16, tag="ones_bf")
        nc.vector.memset(ones_bf[:], 1.0)
        ones_f = pers.tile([P, P], F32, tag="ones_f")
        nc.vector.memset(ones_f[:], 1.0)
        ones1_f = pers.tile([1, HD], F32, tag="ones1_f")
        nc.vector.memset(ones1_f[:], 1.0)
        eps_t = pers.tile([P, 1], F32, tag="eps_t")
        nc.vector.memset(eps_t[:], EPS)

        def wload(pool, dram_ap, kdim, ndim, tag):
            t = pool.tile([P, kdim // P, ndim], BF16, tag=tag)
            nc.sync.dma_start(t[:], dram_ap.rearrange("(ko ki) n -> ki ko n", ki=P))
            return t

        def fm_ln_stats(lnp, x, x_is_f32):
            with tc.tile_pool(name="ln_ps", bufs=1, space="PSUM") as lps:
                sums_ps = lps.tile([P, R], F32, tag="ln_sums")
                sqs_ps = lps.tile([P, R], F32, tag="ln_sqs")
                for half in range(NHALF):
                    o = half * HALF
                    sq = lnp.tile([P, CC, HALF], BF16, tag="ln_sq")
                    nc.scalar.activation(sq[:], x[:, :, o:o + HALF], AF.Square)
                    if x_is_f32:
                        # bf16 shadow so the sums matmul runs at 1 cyc/row
                        xb = lnp.tile([P, CC, HALF], BF16, tag="ln_xb")
                        nc.vector.tensor_copy(xb[:], x[:, :, o:o + HALF])
                    for kc in range(CC):
                        for nn in range(N512):
                            sl = slice(o + nn * 512, o + (nn + 1) * 512)
                            sli = slice(nn * 512, (nn + 1) * 512)
                            xsrc = xb[:, kc, sli] if x_is_f32 else x[:, kc, sl]
                            nc.tensor.matmul(sums_ps[:, sl], ones_bf[:], xsrc,
                                             start=(kc == 0), stop=(kc == CC - 1))
                            nc.tensor.matmul(sqs_ps[:, sl], ones_bf[:],
                                             sq[:, kc, sli],
                                             start=(kc == 0), stop=(kc == CC - 1))
                neg_m = lnp.tile([P, R], F32, tag="ln_negm")
                nc.vector.tensor_scalar(neg_m[:], sums_ps[:], -1.0 / C, None,
                                        op0=OP.mult)
                msq = lnp.tile([P, R], F32, tag="ln_tmp")
                nc.scalar.activation(msq[:], neg_m[:], AF.Square)
                var = lnp.tile([P, R], F32, tag="ln_tmp2")
                nc.vector.scalar_tensor_tensor(var[:], sqs_ps[:], 1.0 / C, msq[:],
                                               op0=OP.mult, op1=OP.subtract)
            std = lnp.tile([P, R], F32, tag="ln_tmp")
            nc.scalar.activation(std[:], var[:], AF.Sqrt, bias=eps_t[:])
            inv_std = lnp.tile([P, R], F32, tag="ln_istd")
            nc.vector.reciprocal(inv_std[:], std[:])
            return neg_m, inv_std

        def fm_ln_apply(lnp, x, neg_m, inv_std, g, b, out, m, res=None):
            t = lnp.tile([P, R], F32, tag="ln_t")
            nc.vector.tensor_tensor(t[:], x[:, m, :], neg_m[:], op=OP.add)
            nc.vector.tensor_tensor(t[:], t[:], inv_std[:], op=OP.mult)
            if res is None:
                nc.vector.tensor_scalar(out[:, m, :], t[:], g[:, m:m + 1],
                                        b[:, m:m + 1], op0=OP.mult, op1=OP.add)
            else:
                nc.vector.tensor_scalar(t[:], t[:], g[:, m:m + 1], b[:, m:m + 1],
                                        op0=OP.mult, op1=OP.add)
                nc.vector.tensor_tensor(out[:, m, :], t[:], res[:, m, :], op=OP.add)

        feat1 = None

        # =========================== CPE =================================
        with tc.tile_pool(name="hpool", bufs=1) as hpool:
            h1_sb = hpool.tile([P, CC, R], BF16, tag="h1")
            with (
                tc.tile_pool(name="cpe_in", bufs=3) as cin,
                tc.tile_pool(name="cpe_ps", bufs=1, space="PSUM") as cps,
            ):
                for half in range(NHALF):
                    hp = [cps.tile([P, HALF], F32, tag=f"mm_ps{m}", name=f"cpe_ps{m}") for m in range(CC)]
                    for kk in range(NKK):
                        w_t = wload(cin, wcat[kk], C, C, "wkk")
                        gi = cin.tile([P, HALF // 16], I16, tag="gi")
                        nc.sync.dma_start(gi[:], gidx[half * NKK + kk])
                        g_t = cin.tile([P, CC, HALF], BF16, tag="gt")
                        nc.gpsimd.dma_gather(g_t[:], feat_bf[:], gi[:], HALF, HALF,
                                             C, transpose=True, single_packet=False)
                        for kc in range(CC):
                            for m in range(CC):
                                for nn in range(N512):
                                    sl = slice(nn * 512, (nn + 1) * 512)
                                    nc.tensor.matmul(
                                        hp[m][:, sl],
                                        w_t[:, kc, m * P:(m + 1) * P],
                                        g_t[:, kc, sl],
                                        start=(kk == 0 and kc == 0),
                                        stop=(kk == NKK - 1 and kc == CC - 1))
                    for m in range(CC):
                        nc.vector.tensor_scalar(
                            h1_sb[:, m, half * HALF:(half + 1) * HALF], hp[m][:],
                            pv["cpe_b"][:, m:m + 1], None, op0=OP.add)

            h2_sb = hpool.tile([P, CC, R], BF16, tag="h2")
            with (
                tc.tile_pool(name="linw", bufs=1) as lwp,
                tc.tile_pool(name="lin_ps", bufs=1, space="PSUM") as lps,
            ):
                lin_w_t = wload(lwp, lin_wT[:], C, C, "lin_w")
                for half in range(NHALF):
                    hp = [lps.tile([P, HALF], F32, tag=f"mm_ps{m}", name=f"lin_ps{m}") for m in range(CC)]
                    for kc in range(CC):
                        for m in range(CC):
                            for nn in range(N512):
                                sl = slice(nn * 512, (nn + 1) * 512)
                                hsl = slice(half * HALF + nn * 512,
                                            half * HALF + (nn + 1) * 512)
                                nc.tensor.matmul(
                                    hp[m][:, sl],
                                    lin_w_t[:, kc, m * P:(m + 1) * P],
                                    h1_sb[:, kc, hsl],
                                    start=(kc == 0), stop=(kc == CC - 1))
                    for m in range(CC):
                        nc.vector.tensor_scalar(
                            h2_sb[:, m, half * HALF:(half + 1) * HALF], hp[m][:],
                            pv["lin_b"][:, m:m + 1], None, op0=OP.add)

            feat1 = resid.tile([P, CC, R], F32, tag="resid")
            with tc.tile_pool(name="fown_p", bufs=1) as fp_, \
                 tc.tile_pool(name="lnp1", bufs=1) as lnp:
                neg_m, inv_std = fm_ln_stats(lnp, h2_sb, x_is_f32=False)
                fown = fp_.tile([P, CC, R], F32, tag="fown")
                nc.sync.dma_start(fown[:], featT_own[:])
                for m in range(CC):
                    fm_ln_apply(lnp, h2_sb, neg_m, inv_std, pv["cpe_ln_g"],
                                pv["cpe_ln_b"], feat1, m, res=fown)

        # ===================== ln1 + qkv + attn + proj ===================
        with tc.tile_pool(name="attn", bufs=1) as ap_:
            q_sb = ap_.tile([P, CC, R], BF16, tag="q_sb")
            k_sb = ap_.tile([P, CC, R], BF16, tag="k_sb")
            v_sb = ap_.tile([P, R // P, NH, HD + 1], BF16, tag="v_sb")
            o_sb = ap_.tile([P, CC, R], BF16, tag="o_sb")
            nc.vector.memset(v_sb[:], 1.0)

            with tc.tile_pool(name="x1p", bufs=1) as x1p:
                with tc.tile_pool(name="lnp2", bufs=1) as lnp:
                    neg_m, inv_std = fm_ln_stats(lnp, feat1, x_is_f32=True)
                    x1 = x1p.tile([P, CC, R], BF16, tag="x1")
                    for m in range(CC):
                        fm_ln_apply(lnp, feat1, neg_m, inv_std, pv["ln1_g"],
                                    pv["ln1_b"], x1, m)

                with tc.tile_pool(name="qkv_ps", bufs=1, space="PSUM") as qps, \
                     tc.tile_pool(name="qkvw", bufs=2) as qwp:
                    for half in range(NHALF):
                        o = half * HALF
                        for part, dst, bias, scl in [(0, q_sb, pv["q_b"], SCALE),
                                                     (1, k_sb, pv["k_b"], None)]:
                            qkv_w_t = wload(qwp, qkv_wT[:, part * C:(part + 1) * C],
                                            C, C, "qkv_w_part")
                            pp_ = [qps.tile([P, HALF], F32, tag=f"mm_ps{m}", name=f"qkv_ps{m}")
                                   for m in range(CC)]
                            for kc in range(CC):
                                for m in range(CC):
                                    for nn in range(N512):
                                        sl = slice(nn * 512, (nn + 1) * 512)
                                        nc.tensor.matmul(
                                            pp_[m][:, sl],
                                            qkv_w_t[:, kc, m * P:(m + 1) * P],
                                            x1[:, kc, o + nn * 512:
                                               o + (nn + 1) * 512],
                                            start=(kc == 0), stop=(kc == CC - 1))
                            for m in range(CC):
                                if scl is None:
                                    nc.vector.tensor_scalar(
                                        dst[:, m, o:o + HALF], pp_[m][:],
                                        bias[:, m:m + 1], None, op0=OP.add)
                                else:
                                    nc.vector.tensor_scalar(
                                        dst[:, m, o:o + HALF], pp_[m][:],
                                        bias[:, m:m + 1], scl,
                                        op0=OP.add, op1=OP.mult)
                        v_w_t = wload(qwp, qkv_wT[:, 2 * C:3 * C], C, C,
                                      "qkv_w_part")
                        for rc in range(HALF // P):
                            row0 = o + rc * P
                            vp = qps.tile([P, C], F32, tag=f"mm_ps{rc % 2}",
                                          name="vp")
                            for kc in range(CC):
                                nc.tensor.matmul(
                                    vp[:], x1[:, kc, row0:row0 + P],
                                    v_w_t[:, kc, :],
                                    start=(kc == 0), stop=(kc == CC - 1))
                            nc.vector.tensor_tensor(
                                v_sb[:, row0 // P, :, :HD],
                                vp[:].rearrange("p (h d) -> p h d", d=HD),
                                v_b_t[:].rearrange("p (h d) -> p h d", d=HD),
                                op=OP.add)

            with (
                tc.tile_pool(name="pT_pool", bufs=2) as ptp,
                tc.tile_pool(name="at_ps", bufs=1, space="PSUM") as aps,
                tc.tile_pool(name="at_ps2", bufs=2, space="PSUM") as aps2,
            ):
                for pt in range(NPATCH):
                    po = pt * KP
                    for h in range(NH):
                        hc, hpo = divmod(h * HD, P)
                        pT = ptp.tile([P, KP // P, KP], BF16, tag="pT")
                        for jc in range(KP // P):
                            sps = aps2.tile([P, KP], F32, tag="s_ps")
                            for nn in range(KP // 512):
                                nc.tensor.matmul(
                                    sps[:, nn * 512:(nn + 1) * 512],
                                    k_sb[hpo:hpo + HD, hc,
                                         po + jc * P:po + (jc + 1) * P],
                                    q_sb[hpo:hpo + HD, hc,
                                         po + nn * 512:po + (nn + 1) * 512],
                                    start=True, stop=True)
                            nc.scalar.activation(pT[:, jc, :], sps[:], AF.Exp)
                        ops_ = aps.tile([P, KP], F32, tag="o_ps")
                        for jc in range(KP // P):
                            for nn in range(KP // 512):
                                sl = slice(nn * 512, (nn + 1) * 512)
                                nc.tensor.matmul(
                                    ops_[:HD + 1, sl],
                                    v_sb[:, (po + jc * P) // P, h, :],
                                    pT[:, jc, sl],
                                    start=(jc == 0), stop=(jc == KP // P - 1))
                        # free the O psum bank early: copy to SBUF, then
                        # run the normalize tail off the PE critical path
                        ocp = ptp.tile([HD + 1, KP], F32, tag="ocp")
                        nc.scalar.copy(ocp[:], ops_[:HD + 1, :])
                        rec = ptp.tile([1, KP], F32, tag="rec")
                        nc.vector.reciprocal(rec[:], ocp[HD:HD + 1, :])
                        rps = aps.tile([HD, KP], F32, tag="rec_ps")
                        for nn in range(KP // 512):
                            sl = slice(nn * 512, (nn + 1) * 512)
                            nc.tensor.matmul(rps[:, sl], ones1_f[:], rec[:, sl],
                                             start=True, stop=True)
                        nc.vector.tensor_tensor(
                            o_sb[hpo:hpo + HD, hc, po:po + KP],
                            ocp[:HD, :], rps[:], op=OP.mult)

            feat2 = resid.tile([P, CC, R], F32, tag="resid")
            with (
                tc.tile_pool(name="projw", bufs=1) as pwp,
                tc.tile_pool(name="proj_ps", bufs=1, space="PSUM") as pps,
            ):
                proj_w_t = wload(pwp, proj_wT[:], C, C, "proj_w")
                for half in range(NHALF):
                    o = half * HALF
                    pp_ = [pps.tile([P, HALF], F32, tag=f"mm_ps{m}", name=f"proj_ps{m}")
                           for m in range(CC)]
                    for kc in range(CC):
                        for m in range(CC):
                            for nn in range(N512):
                                sl = slice(nn * 512, (nn + 1) * 512)
                                nc.tensor.matmul(
                                    pp_[m][:, sl],
                                    proj_w_t[:, kc, m * P:(m + 1) * P],
                                    o_sb[:, kc, o + nn * 512:o + (nn + 1) * 512],
                                    start=(kc == 0), stop=(kc == CC - 1))
                    for m in range(CC):
                        nc.vector.scalar_tensor_tensor(
                            feat2[:, m, o:o + HALF], pp_[m][:],
                            pv["proj_b"][:, m:m + 1], feat1[:, m, o:o + HALF],
                            op0=OP.add, op1=OP.add)

        # =============================== MLP =============================
        QH = 512  # row quarter
        with tc.tile_pool(name="mlp", bufs=1) as mp_:
            x2 = mp_.tile([P, CC, R], BF16, tag="x2")
            with tc.tile_pool(name="lnp3", bufs=1) as lnp:
                neg_m, inv_std = fm_ln_stats(lnp, feat2, x_is_f32=True)
                for m in range(CC):
                    fm_ln_apply(lnp, feat2, neg_m, inv_std, pv["ln2_g"],
                                pv["ln2_b"], x2, m)

            gelu_f = AF.Gelu if gelu_exact else AF.Tanh
            fc1_w_t = wload(mp_, fc1_wT[:], C, 4 * C, "fc1_w")
            fc2_w_t = wload(mp_, fc2_wT[:], 4 * C, C, "fc2_w")
            with (
                tc.tile_pool(name="g_pool", bufs=2) as gp_,
                tc.tile_pool(name="out_pool", bufs=2) as op_,
                tc.tile_pool(name="mlp_ps", bufs=2, space="PSUM") as mps,
            ):
                for quarter in range(R // QH):
                    o = quarter * QH
                    g_sb = gp_.tile([P, 4 * CC, QH], BF16, tag="g_sb")
                    for mg in range(4):
                        fp = [mps.tile([P, QH], F32, tag=f"mm_ps{m}", name=f"mlp_ps{m}")
                              for m in range(CC)]
                        for kc in range(CC):
                            for m in range(CC):
                                mm = mg * CC + m
                                nc.tensor.matmul(
                                    fp[m][:],
                                    fc1_w_t[:, kc, mm * P:(mm + 1) * P],
                                    x2[:, kc, o:o + QH],
                                    start=(kc == 0), stop=(kc == CC - 1))
                        for m in range(CC):
                            mm = mg * CC + m
                            nc.scalar.activation(g_sb[:, mm, :], fp[m][:], gelu_f,
                                                 bias=pv["fc1_b"][:, mm:mm + 1])
                    f2 = [mps.tile([P, QH], F32, tag=f"mm_ps{m}", name=f"mlp_ps{m}") for m in range(CC)]
                    for kc in range(4 * CC):
                        for m in range(CC):
                            nc.tensor.matmul(
                                f2[m][:],
                                fc2_w_t[:, kc, m * P:(m + 1) * P],
                                g_sb[:, kc, :],
                                start=(kc == 0), stop=(kc == 4 * CC - 1))
                    out_q = op_.tile([P, CC, QH], F32, tag="out_q")
                    for m in range(CC):
                        nc.vector.scalar_tensor_tensor(
                            out_q[:, m, :], f2[m][:], pv["fc2_b"][:, m:m + 1],
                            feat2[:, m, o:o + QH], op0=OP.add, op1=OP.add)
                    nc.sync.dma_start(outT[:, :, o:o + QH], out_q[:])

    nc.compile()
    return nc


# ====================== host-side preparation ======================

def prep_shared(inputs):
    f32 = np.float32
    bf = ml_dtypes.bfloat16

    def pp(v):
        return np.ascontiguousarray(np.asarray(v, f32).reshape(-1, P).T)

    feat = np.asarray(inputs["feat"], f32)
    feat_bf = np.zeros((NFULL + 1, C), bf)
    feat_bf[:NFULL] = feat.astype(bf)

    qkv_b = np.asarray(inputs["qkv_b"], f32)
    return dict(
        feat_bf=feat_bf,
        wcat=np.ascontiguousarray(
            np.transpose(np.asarray(inputs["cpe_w"], f32), (0, 2, 1))).astype(bf),
        lin_wT=np.ascontiguousarray(np.asarray(inputs["cpe_lin_w"], f32).T).astype(bf),
        qkv_wT=np.ascontiguousarray(np.asarray(inputs["qkv_w"], f32).T).astype(bf),
        proj_wT=np.ascontiguousarray(np.asarray(inputs["proj_w"], f32).T).astype(bf),
        fc1_wT=np.ascontiguousarray(np.asarray(inputs["fc1_w"], f32).T).astype(bf),
        fc2_wT=np.ascontiguousarray(np.asarray(inputs["fc2_w"], f32).T).astype(bf),
        cpe_b=pp(inputs["cpe_b"]), lin_b=pp(inputs["cpe_lin_b"]),
        cpe_ln_g=pp(inputs["cpe_ln_g"]), cpe_ln_b=pp(inputs["cpe_ln_b"]),
        ln1_g=pp(inputs["ln1_g"]), ln1_b=pp(inputs["ln1_b"]),
        ln2_g=pp(inputs["ln2_g"]), ln2_b=pp(inputs["ln2_b"]),
        q_b=pp(qkv_b[:C]), k_b=pp(qkv_b[C:2 * C]),
        v_b_rep=np.ascontiguousarray(np.broadcast_to(qkv_b[2 * C:], (P, C))),
        proj_b=pp(inputs["proj_b"]),
        fc1_b=pp(inputs["fc1_b"]), fc2_b=pp(inputs["fc2_b"]),
    )


def prep_core(inputs, core):
    f32 = np.float32
    order = np.asarray(inputs["order"])
    feat = np.asarray(inputs["feat"], f32)
    nbr = np.asarray(inputs["neighbor_idx"])
    rows = order[core * R:(core + 1) * R]

    featT_own = np.ascontiguousarray(
        feat[rows].T.reshape(CC, P, R).transpose(1, 0, 2))

    nb = nbr[rows].astype(np.int64)
    nb[nb < 0] = NFULL
    gidx = np.zeros((NHALF * NKK, P, HALF // 16), np.int16)
    for half in range(NHALF):
        blk = nb[half * HALF:(half + 1) * HALF]
        for kk in range(NKK):
            w = blk[:, kk].astype(np.int16).reshape(-1, 16).T
            g = gidx[half * NKK + kk]
            for rep in range(P // 16):
                g[rep * 16:(rep + 1) * 16] = w
    return dict(featT_own=featT_own, gidx=gidx), rows


def unshard_out(res_outT):
    return np.ascontiguousarray(
        np.asarray(res_outT).transpose(1, 0, 2).reshape(C, R).T)


# ======================= public entry point =======================

_CACHED_NC = None


def _get_nc():
    global _CACHED_NC
    if _CACHED_NC is None:
        _CACHED_NC = build_program(gelu_exact=True)
    return _CACHED_NC


def kernel(**inputs) -> np.ndarray:
    """Full-input, full-output entry. Shards across 8 NeuronCores by
    serialized patches (2 per core), runs the Bass kernel, scatters the
    per-core outputs back to original point order."""
    from concourse.bass_utils import run_bass_kernel_spmd

    nc = _get_nc()
    inputs = {k: np.asarray(v) for k, v in inputs.items()}
    sh = prep_shared(inputs)
    in_maps, rows_l = [], []
    for c in range(8):
        ci, rows = prep_core(inputs, c)
        in_maps.append({**sh, **ci})
        rows_l.append(rows)

    res = None
    last_err = None
    for attempt in range(3):
        try:
            res = run_bass_kernel_spmd(nc, in_maps, core_ids=list(range(8))).results
            break
        except Exception as e:   # transient NRT/axon hiccups: retry
            last_err = e
            import time as _t
            _t.sleep(2.0)
    if res is None:
        raise last_err

    out = np.zeros((NFULL, C), np.float32)
    for c in range(8):
        out[rows_l[c]] = unshard_out(res[c]["outT"])
    return out

